# revision 31
# baseline (speedup 1.0000x reference)
"""Performer (FAVOR+) attention TRN2 Bass kernel — v4.

Problem: B=4, N=4096, D=1024, H=16, HD=64, M=256 random features.
Sharding: 8 cores = (batch b = c//2) x (sequence half s = c%2).
Each core handles all 16 heads for 2048 query tokens and 2048 k/v
tokens; partial kv/ksum [65,16,256] is AllReduced over seq-half pairs.

v4 changes vs v3 (682us):
  - o_proj hoisted out of the per-tc4 loop into its own dense PE phase
    at the end (the mixed heads+o_proj loop ran PE-sparse and sat at
    the HAM half-clock; a dense 55us matmul burst re-earns 8/8).
  - bq's cq term folded multiplicatively into the stage-D kv_nat copy
    (tensor_scalar mul by exp(cq), exact) so the E-phase exps are
    bias-free and run as ONE [128,2,512] two-bank activation per head.
  - AllReduce split 3 ways (h0-7 / h8-11 / h12-15) with stage D run
    just-in-time per chunk, mostly inside the combined loop, so E
    never waits on a collective.

v3 changes vs v2 (774us):
  - A2 (diag) + stage B (k features/kv) + phase 1q interleaved into
    one loop over head pairs to keep the PE dense (HAM clock at 8/8).
  - kf/qf head-pair matmuls issued back-to-back into disjoint PE row
    strips (0:64 / 64:128) for row-group concurrency.
  - A2 squares on DVE; bk folded into the 1k PSUM->SBUF copy bias.

Math simplifications (exact):
  - q-side max-over-features subtraction skipped (cancels in the
    numerator/denominator ratio; magnitudes stay well inside fp32).
  - max(normalizer, 1e-6) clamp skipped (normalizer >> 1e-6 here).
  - k diag term folded into the k-feature exp bias via mask matmul
    (CDIAG = -0.5 * s^2, s = HD**-0.25), scale s folded into proj.T.
  - 1/sqrt(M) folded into the k-feature exp bias.
  - bq folded into kv_nat/ksum via exp(cq) column scales (exact);
    bv, bo folded host-side: out += bv @ Wo.T + bo.
  - 1/norm computed as exp(-ln(norm)) on ACT.
"""

import math
import sys

import numpy as np

for _p in ("/opt/trn_rl_repo",):
    if _p not in sys.path:
        sys.path.insert(0, _p)

import ml_dtypes
from concourse import bass, tile, mybir
from concourse.bass_utils import run_bass_kernel_spmd

F32 = mybir.dt.float32
BF16 = mybir.dt.bfloat16

B, N, D = 4, 4096, 1024
H, HD, M = 16, 64, 256
NS = 2048  # tokens per core

SNORM = float(HD) ** -0.25
CDIAG = -0.5 * SNORM * SNORM  # -0.0625
EXP_BIAS = -0.5 * math.log(float(M))


def _build():
    nc = bass.Bass(trn_type="TRN2", target_bir_lowering=False, num_devices=8)

    qt = nc.dram_tensor("qt", [128, 8, NS], BF16, kind="ExternalInput")
    kt = nc.dram_tensor("kt", [128, 8, NS], BF16, kind="ExternalInput")
    vt = nc.dram_tensor("vt", [128, 8, NS], BF16, kind="ExternalInput")
    wqt = nc.dram_tensor("wqt", [128, 8, D], BF16, kind="ExternalInput")
    wkt = nc.dram_tensor("wkt", [128, 8, D], BF16, kind="ExternalInput")
    wvt = nc.dram_tensor("wvt", [128, 8, D], BF16, kind="ExternalInput")
    wot = nc.dram_tensor("wot", [128, 8, D], BF16, kind="ExternalInput")
    pjt = nc.dram_tensor("pjt", [HD, M], BF16, kind="ExternalInput")
    mskc = nc.dram_tensor("mskc", [128, 2], BF16, kind="ExternalInput")
    onea = nc.dram_tensor("onea", [128, 64], mybir.dt.float32r,
                          kind="ExternalInput")
    bkc = nc.dram_tensor("bkc", [128, 8], F32, kind="ExternalInput")
    ecqt = nc.dram_tensor("ecqt", [128, 2, 16], F32, kind="ExternalInput")
    idn = nc.dram_tensor("idn", [128, 128], BF16, kind="ExternalInput")
    out = nc.dram_tensor("out", [NS, D], BF16, kind="ExternalOutput")

    mult = mybir.AluOpType.mult
    Exp = mybir.ActivationFunctionType.Exp
    Ln = mybir.ActivationFunctionType.Ln
    Ident = mybir.ActivationFunctionType.Identity

    with tile.TileContext(nc) as tc:
        # ---------------- persistent constants ----------------
        pj2, free_pj2 = tc.tile([128, M], BF16, name="pj2")    # proj.T * s, both halves
        ones2, free_ones2 = tc.tile([128, 64], mybir.dt.float32r, name="ones2")
        idn_sb, free_idn = tc.tile([128, 128], BF16, name="idn_sb")
        bk_sb, free_bk = tc.tile([128, 8], F32, name="bk_sb")
        ecq_sb, free_ecq = tc.tile([128, 2, 16], F32, name="ecq_sb")
        ebias, free_ebias = tc.tile([128, 1], F32, name="ebias")

        nc.sync.dma_start(pj2[0:64, :], pjt[:, :])
        nc.sync.dma_start(pj2[64:128, :], pjt[:, :])
        nc.sync.dma_start(idn_sb[:, :], idn[:, :])
        nc.sync.dma_start(ones2[:, :], onea[:, :])
        nc.sync.dma_start(bk_sb[:, :], bkc[:, :])
        nc.sync.dma_start(ecq_sb[:, :, :], ecqt[:, :, :])
        nc.vector.memset(ebias[:, :], EXP_BIAS)

        # ---------------- long-lived buffers (alloc order = LIFO) --------
        qhT, free_qhT = tc.tile([128, 8, NS], BF16, name="qhT")
        kvr_sb, free_kvr = tc.tile([65, 16, M], BF16, name="kvr_sb")
        kv_nat, free_kv_nat = tc.tile([128, 16, 2, 65], BF16, name="kv_nat")
        wo_sb, free_wo = tc.tile([128, 8, D], BF16, name="wo_sb")
        # v_nat[p, tcc, h, 0:64] = vh[tcc*128+p, h*64+d]; col 64 = 1.0
        v_nat, free_v_nat = tc.tile([128, 16, 16, 65], BF16, name="v_nat")
        # khT[p, dc, t] = kh[t, dc*128+p]  (head h at rows (h%2)*64, slot h//2)
        khT, free_khT = tc.tile([128, 8, NS], BF16, name="khT")

        # col 64 of v_nat = 1.0 (the ksum row trick); Pool memset beats a
        # 4096-descriptor scatter DMA.
        nc.gpsimd.memset(v_nat[:, :, :, 64:65], 1.0)

        # ------- phases 1v + 1k: vh = v @ Wv.T, khT = Wk @ k.T + bk ------
        # One block so wk/wq weight loads stream during the 1v matmuls
        # (no PE gap at the phase boundary).
        with tc.tile_pool(name="wv", bufs=1) as wvpool, \
             tc.tile_pool(name="wk", bufs=1) as wkpool, \
             tc.tile_pool(name="vin", bufs=2) as vinpool, \
             tc.tile_pool(name="kin", bufs=2) as kinpool, \
             tc.tile_pool(name="pv", bufs=2, space="PSUM") as pvpool, \
             tc.tile_pool(name="pk", bufs=2, space="PSUM") as pkpool:
            wv_sb = wvpool.tile([128, 8, D], BF16, name="wv_sb")
            wk_sb = wkpool.tile([128, 8, D], BF16, name="wk_sb")
            for _ic in range(8):
                nc.sync.dma_start(wv_sb[:, _ic, :], wvt[:, _ic, :])
            kt_tiles = {}
            kt_tiles[0] = kinpool.tile([128, 8, 512], BF16, name="kt_in")
            nc.sync.dma_start(kt_tiles[0][:, :, :], kt[:, :, 0:512])
            for _ic in range(8):
                nc.sync.dma_start(wk_sb[:, _ic, :], wkt[:, _ic, :])
            for tcc in range(16):
                vt_in = vinpool.tile([128, 8, 128], BF16, name="vt_in")
                nc.sync.dma_start(vt_in[:, :, :], vt[:, :, tcc * 128:(tcc + 1) * 128])
                for dc in range(2):
                    ps = pvpool.tile([128, 8, 64], F32, name="pv")
                    for ic in range(8):
                        nc.tensor.matmul(
                            ps[:, :, :],
                            vt_in[:, ic, :],
                            wv_sb[:, ic, dc * 512:(dc + 1) * 512],
                            start=(ic == 0), stop=(ic == 7),
                        )
                    nc.scalar.copy(v_nat[:, tcc, dc * 8:(dc + 1) * 8, 0:64],
                                   ps[:, :, :])
            for tcc in range(4):
                kt_in = kt_tiles[tcc]
                if tcc + 1 < 4:
                    kt_tiles[tcc + 1] = kinpool.tile([128, 8, 512], BF16,
                                                     name="kt_in")
                    nc.sync.dma_start(kt_tiles[tcc + 1][:, :, :],
                                      kt[:, :, (tcc + 1) * 512:(tcc + 2) * 512])
                for dc in range(8):
                    ps = pkpool.tile([128, 512], F32, name="pk")
                    for ic in range(8):
                        nc.tensor.matmul(
                            ps[:, :],
                            wk_sb[:, ic, dc * 128:(dc + 1) * 128],
                            kt_in[:, ic, :],
                            start=(ic == 0), stop=(ic == 7),
                        )
                    nc.scalar.activation(
                        khT[:, dc, tcc * 512:(tcc + 1) * 512], ps[:, :],
                        Ident, bias=bk_sb[:, dc:dc + 1])

        # ------- combined loop: A2 (diag) + B (k feats/kv) + 1q + D -----
        # 1q matmul groups are interleaved to keep the PE dense while
        # ACT burns through the k-feature exps; the kv AllReduce runs
        # in three chunks with stage D (transpose + exp(cq) scale)
        # following each chunk just-in-time.
        dpe, free_dpe = tc.tile([128, 8, 16, 2], F32, name="dpe")
        kvT_stage, free_kvT = tc.tile([65, 16, M], BF16, name="kvT_stage")
        dramb_ctx = tc.tile_pool(name="dramb", bufs=1, space="DRAM")
        dramb = dramb_ctx.__enter__()
        cin1 = dramb.tile([65, 8, M], BF16, name="cin1")
        cout1 = dramb.tile([65, 8, M], BF16, name="cout1")
        cin2 = dramb.tile([65, 4, M], BF16, name="cin2")
        cout2 = dramb.tile([65, 4, M], BF16, name="cout2")
        cin3 = dramb.tile([65, 4, M], BF16, name="cin3")
        cout3 = dramb.tile([65, 4, M], BF16, name="cout3")

        RG = [[0, 1], [2, 3], [4, 5], [6, 7]]

        def stage_d(h, ptppool):
            # kvr chunk -> natural layout, scaled by exp(cq) (folds bq)
            for fc in range(2):
                tp = ptppool.tile([128, 128], BF16, name="tp")
                nc.tensor.transpose(tp[0:128, 0:65],
                                    kvr_sb[0:65, h, fc * 128:(fc + 1) * 128],
                                    idn_sb[0:65, 0:65])
                nc.vector.tensor_scalar(
                    kv_nat[:, h, fc, :], tp[0:128, 0:65],
                    ecq_sb[:, fc, h:h + 1], None, op0=mult)

        with tc.tile_pool(name="msk", bufs=1) as mskpool, \
             tc.tile_pool(name="wq", bufs=1) as wqpool, \
             tc.tile_pool(name="sq2", bufs=2) as sq2pool, \
             tc.tile_pool(name="k2", bufs=2) as k2pool, \
             tc.tile_pool(name="qin", bufs=2) as qinpool, \
             tc.tile_pool(name="pdp", bufs=1, space="PSUM") as pdppool, \
             tc.tile_pool(name="pkf", bufs=2, space="PSUM") as pkfpool, \
             tc.tile_pool(name="pkv", bufs=2, space="PSUM") as pkvpool, \
             tc.tile_pool(name="pq", bufs=2, space="PSUM") as pqpool, \
             tc.tile_pool(name="ptp", bufs=1, space="PSUM") as ptppool:
            mask2 = mskpool.tile([128, 2], BF16, name="mask2")
            nc.sync.dma_start(mask2[:, :], mskc[:, :])
            wq_sb = wqpool.tile([128, 8, D], BF16, name="wq_sb")
            for _ic in range(8):
                nc.sync.dma_start(wq_sb[:, _ic, :], wqt[:, _ic, :])
            for _ic in range(8):
                nc.sync.dma_start(wo_sb[:, _ic, :], wot[:, _ic, :])

            qt_tiles = {}

            def q_group(g):
                # 1q group g in (0..31): tcc4 = g//8, dc = g%8
                tcc4, dc = divmod(g, 8)
                if g % 8 == 0:
                    qt_in = qinpool.tile([128, 8, 512], BF16, name="qt_in")
                    nc.sync.dma_start(qt_in[:, :, :],
                                      qt[:, :, tcc4 * 512:(tcc4 + 1) * 512])
                    qt_tiles[tcc4] = qt_in
                qt_in = qt_tiles[tcc4]
                ps = pqpool.tile([128, 512], F32, name="pq")
                for ic in range(8):
                    nc.tensor.matmul(
                        ps[:, :],
                        wq_sb[:, ic, dc * 128:(dc + 1) * 128],
                        qt_in[:, ic, :],
                        start=(ic == 0), stop=(ic == 7),
                    )
                nc.vector.tensor_copy(
                    qhT[:, dc, tcc4 * 512:(tcc4 + 1) * 512], ps[:, :])

            for pair in range(8):
                # --- A2 for this pair ---
                dps = pdppool.tile([128, 16, 2], F32, name="dps")
                for tcc in range(16):
                    src = khT[:, pair, tcc * 128:(tcc + 1) * 128]
                    sq = sq2pool.tile([128, 128], BF16, name="sq")
                    nc.vector.tensor_tensor(sq[:, :], src, src, mult)
                    nc.tensor.matmul(dps[:, tcc, :], sq[:, :], mask2[:, :],
                                     start=True, stop=True)
                nc.vector.tensor_scalar_add(
                    dpe[:, pair, :, :], dps[:, :, :], ebias[:, 0:1])

                # --- B for heads 2*pair, 2*pair+1 (row-strip paired) ---
                h0, h1 = 2 * pair, 2 * pair + 1
                kvp0 = pkvpool.tile([128, M], F32, name="kvp")
                kvp1 = pkvpool.tile([128, M], F32, name="kvp")
                for tcc in range(16):
                    kf0 = pkfpool.tile([128, M], F32, name="kf")
                    kf1 = pkfpool.tile([128, M], F32, name="kf")
                    nc.tensor.matmul(
                        kf0[:, :], khT[0:64, pair, tcc * 128:(tcc + 1) * 128],
                        pj2[0:64, :], start=True, stop=True)
                    nc.tensor.matmul(
                        kf1[:, :], khT[64:128, pair, tcc * 128:(tcc + 1) * 128],
                        pj2[64:128, :], start=True, stop=True)
                    k20 = k2pool.tile([128, M], BF16, name="k2")
                    k21 = k2pool.tile([128, M], BF16, name="k2")
                    nc.scalar.activation(k20[:, :], kf0[:, :], Exp,
                                         bias=dpe[:, pair, tcc, 0:1])
                    nc.scalar.activation(k21[:, :], kf1[:, :], Exp,
                                         bias=dpe[:, pair, tcc, 1:2])
                    nc.tensor.matmul(kvp0[0:65, :], v_nat[:, tcc, h0, 0:65],
                                     k20[:, :],
                                     start=(tcc == 0), stop=(tcc == 15))
                    nc.tensor.matmul(kvp1[0:65, :], v_nat[:, tcc, h1, 0:65],
                                     k21[:, :],
                                     start=(tcc == 0), stop=(tcc == 15))
                    # spread the 1q groups through the tcc loop so the PE
                    # stays saturated while ACT works through the exps
                    if tcc % 4 == 3:
                        q_group(4 * pair + tcc // 4)
                nc.vector.tensor_copy(kvT_stage[0:65, h0, :], kvp0[0:65, :])
                nc.vector.tensor_copy(kvT_stage[0:65, h1, :], kvp1[0:65, :])

                if pair == 3:
                    nc.gpsimd.dma_start(cin1[:, :, :], kvT_stage[0:65, 0:8, :])
                    nc.gpsimd.collective_compute(
                        "AllReduce", mybir.AluOpType.add,
                        replica_groups=RG, ins=[cin1.opt()], outs=[cout1.opt()])
                elif pair == 5:
                    nc.gpsimd.dma_start(cin2[:, :, :], kvT_stage[0:65, 8:12, :])
                    nc.gpsimd.collective_compute(
                        "AllReduce", mybir.AluOpType.add,
                        replica_groups=RG, ins=[cin2.opt()], outs=[cout2.opt()])
                    # AR1 has landed by now: stage D for heads 0-7
                    nc.sync.dma_start(kvr_sb[0:65, 0:8, :], cout1[:, :, :])
                    for h in range(8):
                        stage_d(h, ptppool)
                elif pair == 7:
                    nc.gpsimd.dma_start(cin3[:, :, :], kvT_stage[0:65, 12:16, :])
                    nc.gpsimd.collective_compute(
                        "AllReduce", mybir.AluOpType.add,
                        replica_groups=RG, ins=[cin3.opt()], outs=[cout3.opt()])
                    nc.sync.dma_start(kvr_sb[0:65, 8:12, :], cout2[:, :, :])
                    for h in range(8, 12):
                        stage_d(h, ptppool)

            nc.sync.dma_start(kvr_sb[0:65, 12:16, :], cout3[:, :, :])

        free_kvT()
        free_dpe()
        free_khT()
        free_v_nat()
        # reuses the khT/v_nat SBUF space
        attn_all, free_attn = tc.tile([128, 4, 8, 512], BF16, name="attn_all")

        # ---------------- stage E: q features, attention ----------
        with tc.tile_pool(name="qp", bufs=3) as qppool, \
             tc.tile_pool(name="lnr", bufs=3) as lnrpool, \
             tc.tile_pool(name="rin", bufs=3) as rinpool, \
             tc.tile_pool(name="aodd", bufs=2) as oddpool, \
             tc.tile_pool(name="pqf", bufs=2, space="PSUM") as pqfpool, \
             tc.tile_pool(name="ppo", bufs=3, space="PSUM") as ppopool, \
             tc.tile_pool(name="pbc", bufs=2, space="PSUM") as pbcpool, \
             tc.tile_pool(name="ptp2", bufs=1, space="PSUM") as ptp2pool:
            for tc4 in range(4):
                tsl = slice(tc4 * 512, (tc4 + 1) * 512)
                aodd = oddpool.tile([128, 8, 512], BF16, name="aodd")
                for pair in range(8):
                    if tc4 == 0 and pair == 6:
                        # AR3 has landed: stage D for heads 12-15
                        for h in range(12, 16):
                            stage_d(h, ptp2pool)
                    h0, h1 = 2 * pair, 2 * pair + 1
                    # qf matmuls for the two heads run in disjoint PE
                    # row strips (0:64 / 64:128): issue back-to-back.
                    # Short-lived 1-bank qf tiles (bufs=3) let pair p+1's
                    # q-features proceed while pair p's epilogue drains.
                    qp0 = qppool.tile([128, 2, 512], BF16, name="qp")
                    qp1 = qppool.tile([128, 2, 512], BF16, name="qp")
                    po0 = ppopool.tile([128, 512], F32, name="po65")
                    po1 = ppopool.tile([128, 512], F32, name="po65")
                    for fc in range(2):
                        qf0 = pqfpool.tile([128, 512], F32, name="qf")
                        qf1 = pqfpool.tile([128, 512], F32, name="qf")
                        nc.tensor.matmul(
                            qf0[:, :], pj2[0:64, fc * 128:(fc + 1) * 128],
                            qhT[0:64, pair, tsl], start=True, stop=True)
                        nc.tensor.matmul(
                            qf1[:, :], pj2[64:128, fc * 128:(fc + 1) * 128],
                            qhT[64:128, pair, tsl], start=True, stop=True)
                        nc.scalar.activation(qp0[:, fc, :], qf0[:, :], Exp)
                        nc.scalar.activation(qp1[:, fc, :], qf1[:, :], Exp)
                        nc.tensor.matmul(po0[0:65, :], kv_nat[:, h0, fc, 0:65],
                                         qp0[:, fc, :],
                                         start=(fc == 0), stop=(fc == 1))
                        nc.tensor.matmul(po1[0:65, :], kv_nat[:, h1, fc, 0:65],
                                         qp1[:, fc, :],
                                         start=(fc == 0), stop=(fc == 1))
                    for hh, po65 in ((0, po0), (1, po1)):
                        # 1/norm: ln(norm) on the [1,512] row, PE-broadcast
                        # the log to 64 partitions, then exp(-x) straight
                        # into SBUF bf16 — no [1,512] rexp, no DVE copy.
                        lnr = lnrpool.tile([65, 512], mybir.dt.float32r,
                                           name="lnr")
                        nc.scalar.activation(lnr[64:65, :], po65[64:65, :], Ln)
                        bc = pbcpool.tile([128, 512], F32, name="bc")
                        nc.tensor.matmul(bc[0:64, :], ones2[64:65, 0:64],
                                         lnr[64:65, :], start=True, stop=True)
                        rin = rinpool.tile([128, 512], BF16, name="rin")
                        nc.scalar.activation(rin[0:64, :], bc[0:64, :], Exp,
                                             scale=-1.0)
                        if hh == 0:
                            nc.vector.tensor_tensor(
                                attn_all[0:64, tc4, pair, :],
                                po65[0:64, :], rin[0:64, :], mult)
                        else:
                            nc.vector.tensor_tensor(aodd[0:64, pair, :],
                                                    po65[0:64, :],
                                                    rin[0:64, :], mult)
                            nc.sync.dma_start(attn_all[64:128, tc4, pair, :],
                                              aodd[0:64, pair, :])

        # ---------------- o_proj: dense PE phase ----------------
        with tc.tile_pool(name="po5", bufs=2, space="PSUM") as po5pool, \
             tc.tile_pool(name="osb", bufs=3) as osbpool:
            for tc4 in range(4):
                for tcc in range(4):
                    o_sb = osbpool.tile([128, D], BF16, name="o_sb")
                    for j in range(2):
                        p5 = po5pool.tile([128, 512], F32, name="p5")
                        for pair in range(8):
                            nc.tensor.matmul(
                                p5[:, :],
                                attn_all[:, tc4, pair,
                                         tcc * 128:(tcc + 1) * 128],
                                wo_sb[:, pair, j * 512:(j + 1) * 512],
                                start=(pair == 0), stop=(pair == 7),
                            )
                        nc.vector.tensor_copy(o_sb[:, j * 512:(j + 1) * 512],
                                              p5[:, :])
                    nc.sync.dma_start(
                        out[tc4 * 512 + tcc * 128:tc4 * 512 + (tcc + 1) * 128, :],
                        o_sb[:, :])

        free_attn()
        for f in (free_wo, free_kv_nat, free_kvr, free_qhT):
            f()
        dramb_ctx.__exit__(None, None, None)
        for f in (free_ebias, free_ecq, free_bk, free_idn, free_ones2,
                  free_pj2):
            f()

    # TRN2 walrus codegen allows at most 1 sync wait per instruction
    # (2 on InstEventSemaphore); split excess waits into event semaphores.
    import bass_rust
    bass_rust.generate_event_semaphores(nc)
    return nc


_CACHE = {}


def _get_nc():
    if "nc" not in _CACHE:
        _CACHE["nc"] = _build()
    return _CACHE["nc"]


def _bf16(x):
    return np.ascontiguousarray(x).astype(ml_dtypes.bfloat16)


def _shard(x):
    # [2048, 1024] token-slice -> [128, 8, 2048] with [p, ic, t] = x[t, ic*128+p]
    return _bf16(x.T.reshape(8, 128, NS).transpose(1, 0, 2))


def _wlayout(w):
    # W [D, D] -> [128, 8, D] with [p, ic, d] = W[d, ic*128+p]
    return _bf16(w.T.reshape(8, 128, D).transpose(1, 0, 2))


def _run(nc, in_maps, trace=False, tmpdir=None):
    return run_bass_kernel_spmd(nc, in_maps, list(range(8)), trace=trace,
                                tmpdir=tmpdir)


def kernel(q, k, v, Wq, bq, Wk, bk, Wv, bv, Wo, bo, proj,
           _trace=False, _tmpdir=None):
    nc = _get_nc()

    q = np.asarray(q); k = np.asarray(k); v = np.asarray(v)
    Wq = np.asarray(Wq); Wk = np.asarray(Wk); Wv = np.asarray(Wv)
    Wo = np.asarray(Wo); proj = np.asarray(proj)
    bq = np.asarray(bq); bk = np.asarray(bk); bv = np.asarray(bv)
    bo = np.asarray(bo)

    pjt = _bf16(proj.T * SNORM)
    wqt = _wlayout(Wq)
    wkt = _wlayout(Wk)
    wvt = _wlayout(Wv)
    wot = _wlayout(Wo)
    idn = _bf16(np.eye(128, dtype=np.float32))
    mskc = np.zeros((128, 2), dtype=np.float32)
    mskc[0:64, 0] = CDIAG
    mskc[64:128, 1] = CDIAG
    mskc = _bf16(mskc)
    onea = np.ones((128, 64), dtype=np.float32)
    oneb = _bf16(np.ones((128, 16, 16, 1), dtype=np.float32))
    bkcl = np.ascontiguousarray(bk.reshape(8, 128).T).astype(np.float32)
    # cq[m, h] = s * (bq_h . proj_m); exp(cq) scales kv_nat/ksum (exact)
    cq = (proj @ bq.reshape(H, HD).T) * SNORM          # [M, H]
    ecqt = np.ascontiguousarray(
        np.exp(cq).reshape(2, 128, H).transpose(1, 0, 2)).astype(np.float32)

    in_maps = []
    for c in range(8):
        b, s = divmod(c, 2)
        sl = slice(s * NS, (s + 1) * NS)
        in_maps.append({
            "qt": _shard(q[b, sl, :]),
            "kt": _shard(k[b, sl, :]),
            "vt": _shard(v[b, sl, :]),
            "wqt": wqt, "wkt": wkt, "wvt": wvt, "wot": wot,
            "pjt": pjt, "idn": idn, "mskc": mskc,
            "onea": onea, "oneb": oneb, "bkc": bkcl,
            "ecqt": ecqt,
        })

    res = _run(nc, in_maps, trace=_trace, tmpdir=_tmpdir)

    bo_eff = (bv @ Wo.T + bo).astype(np.float32)
    full = np.empty((B, N, D), dtype=np.float32)
    for c in range(8):
        b, s = divmod(c, 2)
        full[b, s * NS:(s + 1) * NS, :] = \
            res.results[c]["out"].astype(np.float32) + bo_eff

    if _trace:
        return full, res
    return full


# revision 32
# speedup vs baseline: 1.0147x; 1.0147x over previous
"""Performer (FAVOR+) attention TRN2 Bass kernel — v4.

Problem: B=4, N=4096, D=1024, H=16, HD=64, M=256 random features.
Sharding: 8 cores = (batch b = c//2) x (sequence half s = c%2).
Each core handles all 16 heads for 2048 query tokens and 2048 k/v
tokens; partial kv/ksum [65,16,256] is AllReduced over seq-half pairs.

v4 changes vs v3 (682us):
  - o_proj hoisted out of the per-tc4 loop into its own dense PE phase
    at the end (the mixed heads+o_proj loop ran PE-sparse and sat at
    the HAM half-clock; a dense 55us matmul burst re-earns 8/8).
  - bq's cq term folded multiplicatively into the stage-D kv_nat copy
    (tensor_scalar mul by exp(cq), exact) so the E-phase exps are
    bias-free and run as ONE [128,2,512] two-bank activation per head.
  - AllReduce split 3 ways (h0-7 / h8-11 / h12-15) with stage D run
    just-in-time per chunk, mostly inside the combined loop, so E
    never waits on a collective.

v3 changes vs v2 (774us):
  - A2 (diag) + stage B (k features/kv) + phase 1q interleaved into
    one loop over head pairs to keep the PE dense (HAM clock at 8/8).
  - kf/qf head-pair matmuls issued back-to-back into disjoint PE row
    strips (0:64 / 64:128) for row-group concurrency.
  - A2 squares on DVE; bk folded into the 1k PSUM->SBUF copy bias.

Math simplifications (exact):
  - q-side max-over-features subtraction skipped (cancels in the
    numerator/denominator ratio; magnitudes stay well inside fp32).
  - max(normalizer, 1e-6) clamp skipped (normalizer >> 1e-6 here).
  - k diag term folded into the k-feature exp bias via mask matmul
    (CDIAG = -0.5 * s^2, s = HD**-0.25), scale s folded into proj.T.
  - 1/sqrt(M) folded into the k-feature exp bias.
  - bq folded into kv_nat/ksum via exp(cq) column scales (exact);
    bv, bo folded host-side: out += bv @ Wo.T + bo.
  - 1/norm computed as exp(-ln(norm)) on ACT.
"""

import math
import sys

import numpy as np

for _p in ("/opt/trn_rl_repo",):
    if _p not in sys.path:
        sys.path.insert(0, _p)

import ml_dtypes
from concourse import bass, tile, mybir
from concourse.bass_utils import run_bass_kernel_spmd

F32 = mybir.dt.float32
BF16 = mybir.dt.bfloat16

B, N, D = 4, 4096, 1024
H, HD, M = 16, 64, 256
NS = 2048  # tokens per core

SNORM = float(HD) ** -0.25
CDIAG = -0.5 * SNORM * SNORM  # -0.0625
EXP_BIAS = -0.5 * math.log(float(M))


def _build():
    nc = bass.Bass(trn_type="TRN2", target_bir_lowering=False, num_devices=8)

    qt = nc.dram_tensor("qt", [128, 8, NS], BF16, kind="ExternalInput")
    kt = nc.dram_tensor("kt", [128, 8, NS], BF16, kind="ExternalInput")
    vt = nc.dram_tensor("vt", [128, 8, NS], BF16, kind="ExternalInput")
    wqt = nc.dram_tensor("wqt", [128, 8, D], BF16, kind="ExternalInput")
    wkt = nc.dram_tensor("wkt", [128, 8, D], BF16, kind="ExternalInput")
    wvt = nc.dram_tensor("wvt", [128, 8, D], BF16, kind="ExternalInput")
    wot = nc.dram_tensor("wot", [128, 8, D], BF16, kind="ExternalInput")
    pjt = nc.dram_tensor("pjt", [HD, M], BF16, kind="ExternalInput")
    mskc = nc.dram_tensor("mskc", [128, 2], BF16, kind="ExternalInput")
    onea = nc.dram_tensor("onea", [128, 64], BF16, kind="ExternalInput")
    bkc = nc.dram_tensor("bkc", [128, 8], F32, kind="ExternalInput")
    ecqt = nc.dram_tensor("ecqt", [128, 2, 16], F32, kind="ExternalInput")
    idn = nc.dram_tensor("idn", [128, 128], BF16, kind="ExternalInput")
    out = nc.dram_tensor("out", [NS, D], BF16, kind="ExternalOutput")

    mult = mybir.AluOpType.mult
    Exp = mybir.ActivationFunctionType.Exp
    Ln = mybir.ActivationFunctionType.Ln
    Ident = mybir.ActivationFunctionType.Identity

    with tile.TileContext(nc) as tc:
        # ---------------- persistent constants ----------------
        pj2, free_pj2 = tc.tile([128, M], BF16, name="pj2")    # proj.T * s, both halves
        ones2, free_ones2 = tc.tile([128, 64], BF16, name="ones2")
        idn_sb, free_idn = tc.tile([128, 128], BF16, name="idn_sb")
        bk_sb, free_bk = tc.tile([128, 8], F32, name="bk_sb")
        ecq_sb, free_ecq = tc.tile([128, 2, 16], F32, name="ecq_sb")
        ebias, free_ebias = tc.tile([128, 1], F32, name="ebias")

        nc.sync.dma_start(pj2[0:64, :], pjt[:, :])
        nc.sync.dma_start(pj2[64:128, :], pjt[:, :])
        nc.sync.dma_start(idn_sb[:, :], idn[:, :])
        nc.sync.dma_start(ones2[:, :], onea[:, :])
        nc.sync.dma_start(bk_sb[:, :], bkc[:, :])
        nc.sync.dma_start(ecq_sb[:, :, :], ecqt[:, :, :])
        nc.vector.memset(ebias[:, :], EXP_BIAS)

        # ---------------- long-lived buffers (alloc order = LIFO) --------
        qhT, free_qhT = tc.tile([128, 8, NS], BF16, name="qhT")
        kvr_sb, free_kvr = tc.tile([65, 16, M], BF16, name="kvr_sb")
        kv_nat, free_kv_nat = tc.tile([128, 16, 2, 65], BF16, name="kv_nat")
        wo_sb, free_wo = tc.tile([128, 8, D], BF16, name="wo_sb")
        # v_nat[p, tcc, h, 0:64] = vh[tcc*128+p, h*64+d]; col 64 = 1.0
        v_nat, free_v_nat = tc.tile([128, 16, 16, 65], BF16, name="v_nat")
        # khT[p, dc, t] = kh[t, dc*128+p]  (head h at rows (h%2)*64, slot h//2)
        khT, free_khT = tc.tile([128, 8, NS], BF16, name="khT")

        # col 64 of v_nat = 1.0 (the ksum row trick); Pool memset beats a
        # 4096-descriptor scatter DMA.
        nc.gpsimd.memset(v_nat[:, :, :, 64:65], 1.0)

        # ------- phases 1v + 1k: vh = v @ Wv.T, khT = Wk @ k.T + bk ------
        # One block so wk/wq weight loads stream during the 1v matmuls
        # (no PE gap at the phase boundary).
        with tc.tile_pool(name="wv", bufs=1) as wvpool, \
             tc.tile_pool(name="wk", bufs=1) as wkpool, \
             tc.tile_pool(name="vin", bufs=2) as vinpool, \
             tc.tile_pool(name="kin", bufs=2) as kinpool, \
             tc.tile_pool(name="pv", bufs=2, space="PSUM") as pvpool, \
             tc.tile_pool(name="pk", bufs=2, space="PSUM") as pkpool:
            wv_sb = wvpool.tile([128, 8, D], BF16, name="wv_sb")
            wk_sb = wkpool.tile([128, 8, D], BF16, name="wk_sb")
            for _ic in range(8):
                nc.sync.dma_start(wv_sb[:, _ic, :], wvt[:, _ic, :])
            kt_tiles = {}
            kt_tiles[0] = kinpool.tile([128, 8, 512], BF16, name="kt_in")
            nc.sync.dma_start(kt_tiles[0][:, :, :], kt[:, :, 0:512])
            for _ic in range(8):
                nc.sync.dma_start(wk_sb[:, _ic, :], wkt[:, _ic, :])
            for tcc in range(16):
                vt_in = vinpool.tile([128, 8, 128], BF16, name="vt_in")
                nc.sync.dma_start(vt_in[:, :, :], vt[:, :, tcc * 128:(tcc + 1) * 128])
                for dc in range(2):
                    ps = pvpool.tile([128, 8, 64], F32, name="pv")
                    for ic in range(8):
                        nc.tensor.matmul(
                            ps[:, :, :],
                            vt_in[:, ic, :],
                            wv_sb[:, ic, dc * 512:(dc + 1) * 512],
                            start=(ic == 0), stop=(ic == 7),
                        )
                    nc.scalar.copy(v_nat[:, tcc, dc * 8:(dc + 1) * 8, 0:64],
                                   ps[:, :, :])
            for tcc in range(4):
                kt_in = kt_tiles[tcc]
                if tcc + 1 < 4:
                    kt_tiles[tcc + 1] = kinpool.tile([128, 8, 512], BF16,
                                                     name="kt_in")
                    nc.sync.dma_start(kt_tiles[tcc + 1][:, :, :],
                                      kt[:, :, (tcc + 1) * 512:(tcc + 2) * 512])
                for dc in range(8):
                    ps = pkpool.tile([128, 512], F32, name="pk")
                    for ic in range(8):
                        nc.tensor.matmul(
                            ps[:, :],
                            wk_sb[:, ic, dc * 128:(dc + 1) * 128],
                            kt_in[:, ic, :],
                            start=(ic == 0), stop=(ic == 7),
                        )
                    nc.scalar.activation(
                        khT[:, dc, tcc * 512:(tcc + 1) * 512], ps[:, :],
                        Ident, bias=bk_sb[:, dc:dc + 1])

        # ------- combined loop: A2 (diag) + B (k feats/kv) + 1q + D -----
        # 1q matmul groups are interleaved to keep the PE dense while
        # ACT burns through the k-feature exps; the kv AllReduce runs
        # in three chunks with stage D (transpose + exp(cq) scale)
        # following each chunk just-in-time.
        dpe, free_dpe = tc.tile([128, 8, 16, 2], F32, name="dpe")
        kvT_stage, free_kvT = tc.tile([65, 16, M], BF16, name="kvT_stage")
        dramb_ctx = tc.tile_pool(name="dramb", bufs=1, space="DRAM")
        dramb = dramb_ctx.__enter__()
        cin1 = dramb.tile([65, 8, M], BF16, name="cin1")
        cout1 = dramb.tile([65, 8, M], BF16, name="cout1")
        cin2 = dramb.tile([65, 4, M], BF16, name="cin2")
        cout2 = dramb.tile([65, 4, M], BF16, name="cout2")
        cin3 = dramb.tile([65, 4, M], BF16, name="cin3")
        cout3 = dramb.tile([65, 4, M], BF16, name="cout3")

        RG = [[0, 1], [2, 3], [4, 5], [6, 7]]

        def stage_d(h, ptppool):
            # kvr chunk -> natural layout, scaled by exp(cq) (folds bq)
            for fc in range(2):
                tp = ptppool.tile([128, 128], BF16, name="tp")
                nc.tensor.transpose(tp[0:128, 0:65],
                                    kvr_sb[0:65, h, fc * 128:(fc + 1) * 128],
                                    idn_sb[0:65, 0:65])
                nc.vector.tensor_scalar(
                    kv_nat[:, h, fc, :], tp[0:128, 0:65],
                    ecq_sb[:, fc, h:h + 1], None, op0=mult)

        with tc.tile_pool(name="msk", bufs=1) as mskpool, \
             tc.tile_pool(name="wq", bufs=1) as wqpool, \
             tc.tile_pool(name="sq2", bufs=2) as sq2pool, \
             tc.tile_pool(name="k2", bufs=2) as k2pool, \
             tc.tile_pool(name="qin", bufs=2) as qinpool, \
             tc.tile_pool(name="pdp", bufs=1, space="PSUM") as pdppool, \
             tc.tile_pool(name="pkf", bufs=2, space="PSUM") as pkfpool, \
             tc.tile_pool(name="pkv", bufs=2, space="PSUM") as pkvpool, \
             tc.tile_pool(name="pq", bufs=2, space="PSUM") as pqpool, \
             tc.tile_pool(name="ptp", bufs=1, space="PSUM") as ptppool:
            mask2 = mskpool.tile([128, 2], BF16, name="mask2")
            nc.sync.dma_start(mask2[:, :], mskc[:, :])
            wq_sb = wqpool.tile([128, 8, D], BF16, name="wq_sb")
            for _ic in range(8):
                nc.sync.dma_start(wq_sb[:, _ic, :], wqt[:, _ic, :])
            for _ic in range(8):
                nc.sync.dma_start(wo_sb[:, _ic, :], wot[:, _ic, :])

            qt_tiles = {}

            def q_group(g):
                # 1q group g in (0..31): tcc4 = g//8, dc = g%8
                tcc4, dc = divmod(g, 8)
                if g % 8 == 0:
                    qt_in = qinpool.tile([128, 8, 512], BF16, name="qt_in")
                    nc.sync.dma_start(qt_in[:, :, :],
                                      qt[:, :, tcc4 * 512:(tcc4 + 1) * 512])
                    qt_tiles[tcc4] = qt_in
                qt_in = qt_tiles[tcc4]
                ps = pqpool.tile([128, 512], F32, name="pq")
                for ic in range(8):
                    nc.tensor.matmul(
                        ps[:, :],
                        wq_sb[:, ic, dc * 128:(dc + 1) * 128],
                        qt_in[:, ic, :],
                        start=(ic == 0), stop=(ic == 7),
                    )
                nc.vector.tensor_copy(
                    qhT[:, dc, tcc4 * 512:(tcc4 + 1) * 512], ps[:, :])

            for pair in range(8):
                # --- A2 for this pair ---
                dps = pdppool.tile([128, 16, 2], F32, name="dps")
                for tcc in range(16):
                    src = khT[:, pair, tcc * 128:(tcc + 1) * 128]
                    sq = sq2pool.tile([128, 128], BF16, name="sq")
                    nc.vector.tensor_tensor(sq[:, :], src, src, mult)
                    nc.tensor.matmul(dps[:, tcc, :], sq[:, :], mask2[:, :],
                                     start=True, stop=True)
                nc.vector.tensor_scalar_add(
                    dpe[:, pair, :, :], dps[:, :, :], ebias[:, 0:1])

                # --- B for heads 2*pair, 2*pair+1 (row-strip paired) ---
                h0, h1 = 2 * pair, 2 * pair + 1
                kvp0 = pkvpool.tile([128, M], F32, name="kvp")
                kvp1 = pkvpool.tile([128, M], F32, name="kvp")
                for tcc in range(16):
                    kf0 = pkfpool.tile([128, M], F32, name="kf")
                    kf1 = pkfpool.tile([128, M], F32, name="kf")
                    nc.tensor.matmul(
                        kf0[:, :], khT[0:64, pair, tcc * 128:(tcc + 1) * 128],
                        pj2[0:64, :], start=True, stop=True)
                    nc.tensor.matmul(
                        kf1[:, :], khT[64:128, pair, tcc * 128:(tcc + 1) * 128],
                        pj2[64:128, :], start=True, stop=True)
                    k20 = k2pool.tile([128, M], BF16, name="k2")
                    k21 = k2pool.tile([128, M], BF16, name="k2")
                    nc.scalar.activation(k20[:, :], kf0[:, :], Exp,
                                         bias=dpe[:, pair, tcc, 0:1])
                    nc.scalar.activation(k21[:, :], kf1[:, :], Exp,
                                         bias=dpe[:, pair, tcc, 1:2])
                    nc.tensor.matmul(kvp0[0:65, :], v_nat[:, tcc, h0, 0:65],
                                     k20[:, :],
                                     start=(tcc == 0), stop=(tcc == 15))
                    nc.tensor.matmul(kvp1[0:65, :], v_nat[:, tcc, h1, 0:65],
                                     k21[:, :],
                                     start=(tcc == 0), stop=(tcc == 15))
                    # spread the 1q groups through the tcc loop so the PE
                    # stays saturated while ACT works through the exps
                    if tcc % 4 == 3:
                        q_group(4 * pair + tcc // 4)
                nc.vector.tensor_copy(kvT_stage[0:65, h0, :], kvp0[0:65, :])
                nc.vector.tensor_copy(kvT_stage[0:65, h1, :], kvp1[0:65, :])

                if pair == 3:
                    nc.gpsimd.dma_start(cin1[:, :, :], kvT_stage[0:65, 0:8, :])
                    nc.gpsimd.collective_compute(
                        "AllReduce", mybir.AluOpType.add,
                        replica_groups=RG, ins=[cin1.opt()], outs=[cout1.opt()])
                elif pair == 5:
                    nc.gpsimd.dma_start(cin2[:, :, :], kvT_stage[0:65, 8:12, :])
                    nc.gpsimd.collective_compute(
                        "AllReduce", mybir.AluOpType.add,
                        replica_groups=RG, ins=[cin2.opt()], outs=[cout2.opt()])
                    # AR1 has landed by now: stage D for heads 0-7
                    nc.sync.dma_start(kvr_sb[0:65, 0:8, :], cout1[:, :, :])
                    for h in range(8):
                        stage_d(h, ptppool)
                elif pair == 7:
                    nc.gpsimd.dma_start(cin3[:, :, :], kvT_stage[0:65, 12:16, :])
                    nc.gpsimd.collective_compute(
                        "AllReduce", mybir.AluOpType.add,
                        replica_groups=RG, ins=[cin3.opt()], outs=[cout3.opt()])
                    nc.sync.dma_start(kvr_sb[0:65, 8:12, :], cout2[:, :, :])
                    for h in range(8, 12):
                        stage_d(h, ptppool)

            nc.sync.dma_start(kvr_sb[0:65, 12:16, :], cout3[:, :, :])

        free_kvT()
        free_dpe()
        free_khT()
        free_v_nat()
        # reuses the khT/v_nat SBUF space
        attn_all, free_attn = tc.tile([128, 4, 8, 512], BF16, name="attn_all")

        # ---------------- stage E: q features, attention ----------
        with tc.tile_pool(name="qp", bufs=3) as qppool, \
             tc.tile_pool(name="lnr", bufs=3) as lnrpool, \
             tc.tile_pool(name="rin", bufs=3) as rinpool, \
             tc.tile_pool(name="aodd", bufs=2) as oddpool, \
             tc.tile_pool(name="pqf", bufs=2, space="PSUM") as pqfpool, \
             tc.tile_pool(name="ppo", bufs=3, space="PSUM") as ppopool, \
             tc.tile_pool(name="pbc", bufs=2, space="PSUM") as pbcpool, \
             tc.tile_pool(name="ptp2", bufs=1, space="PSUM") as ptp2pool, \
             tc.tile_pool(name="bcs", bufs=3) as bcspool:
            for tc4 in range(4):
                tsl = slice(tc4 * 512, (tc4 + 1) * 512)
                aodd = oddpool.tile([128, 8, 512], BF16, name="aodd")
                for pair in range(8):
                    if tc4 == 0 and pair == 6:
                        # AR3 has landed: stage D for heads 12-15
                        for h in range(12, 16):
                            stage_d(h, ptp2pool)
                    h0, h1 = 2 * pair, 2 * pair + 1
                    # qf matmuls for the two heads run in disjoint PE
                    # row strips (0:64 / 64:128): issue back-to-back.
                    # Short-lived 1-bank qf tiles (bufs=3) let pair p+1's
                    # q-features proceed while pair p's epilogue drains.
                    qp0 = qppool.tile([128, 2, 512], BF16, name="qp")
                    qp1 = qppool.tile([128, 2, 512], BF16, name="qp")
                    po0 = ppopool.tile([128, 512], F32, name="po65")
                    po1 = ppopool.tile([128, 512], F32, name="po65")
                    for fc in range(2):
                        qf0 = pqfpool.tile([128, 512], F32, name="qf")
                        qf1 = pqfpool.tile([128, 512], F32, name="qf")
                        nc.tensor.matmul(
                            qf0[:, :], pj2[0:64, fc * 128:(fc + 1) * 128],
                            qhT[0:64, pair, tsl], start=True, stop=True)
                        nc.tensor.matmul(
                            qf1[:, :], pj2[64:128, fc * 128:(fc + 1) * 128],
                            qhT[64:128, pair, tsl], start=True, stop=True)
                        nc.scalar.activation(qp0[:, fc, :], qf0[:, :], Exp)
                        nc.scalar.activation(qp1[:, fc, :], qf1[:, :], Exp)
                        nc.tensor.matmul(po0[0:65, :], kv_nat[:, h0, fc, 0:65],
                                         qp0[:, fc, :],
                                         start=(fc == 0), stop=(fc == 1))
                        nc.tensor.matmul(po1[0:65, :], kv_nat[:, h1, fc, 0:65],
                                         qp1[:, fc, :],
                                         start=(fc == 0), stop=(fc == 1))
                    for hh, po65 in ((0, po0), (1, po1)):
                        # 1/norm via exp(-ln(norm)) on ACT (row 64)
                        lnr = lnrpool.tile([65, 512], F32, name="lnr")
                        rin = rinpool.tile([65, 512], BF16, name="rin")
                        nc.scalar.activation(lnr[64:65, :], po65[64:65, :], Ln)
                        nc.scalar.activation(rin[64:65, :], lnr[64:65, :], Exp,
                                             scale=-1.0)
                        bc = pbcpool.tile([128, 512], F32, name="bc")
                        nc.tensor.matmul(bc[0:64, :], ones2[64:65, 0:64],
                                         rin[64:65, :], start=True, stop=True)
                        bcs = bcspool.tile([128, 512], BF16, name="bcs")
                        nc.vector.tensor_copy(bcs[0:64, :], bc[0:64, :])
                        if hh == 0:
                            nc.vector.tensor_tensor(
                                attn_all[0:64, tc4, pair, :],
                                po65[0:64, :], bcs[0:64, :], mult)
                        else:
                            nc.vector.tensor_tensor(aodd[0:64, pair, :],
                                                    po65[0:64, :],
                                                    bcs[0:64, :], mult)
                            nc.sync.dma_start(attn_all[64:128, tc4, pair, :],
                                              aodd[0:64, pair, :])

        # ---------------- o_proj: dense PE phase ----------------
        with tc.tile_pool(name="po5", bufs=2, space="PSUM") as po5pool, \
             tc.tile_pool(name="osb", bufs=3) as osbpool:
            for tc4 in range(4):
                for tcc in range(4):
                    o_sb = osbpool.tile([128, D], BF16, name="o_sb")
                    for j in range(2):
                        p5 = po5pool.tile([128, 512], F32, name="p5")
                        for pair in range(8):
                            nc.tensor.matmul(
                                p5[:, :],
                                attn_all[:, tc4, pair,
                                         tcc * 128:(tcc + 1) * 128],
                                wo_sb[:, pair, j * 512:(j + 1) * 512],
                                start=(pair == 0), stop=(pair == 7),
                            )
                        nc.vector.tensor_copy(o_sb[:, j * 512:(j + 1) * 512],
                                              p5[:, :])
                    nc.sync.dma_start(
                        out[tc4 * 512 + tcc * 128:tc4 * 512 + (tcc + 1) * 128, :],
                        o_sb[:, :])

        free_attn()
        for f in (free_wo, free_kv_nat, free_kvr, free_qhT):
            f()
        dramb_ctx.__exit__(None, None, None)
        for f in (free_ebias, free_ecq, free_bk, free_idn, free_ones2,
                  free_pj2):
            f()

    # TRN2 walrus codegen allows at most 1 sync wait per instruction
    # (2 on InstEventSemaphore); split excess waits into event semaphores.
    import bass_rust
    bass_rust.generate_event_semaphores(nc)
    return nc


_CACHE = {}


def _get_nc():
    if "nc" not in _CACHE:
        _CACHE["nc"] = _build()
    return _CACHE["nc"]


def _bf16(x):
    return np.ascontiguousarray(x).astype(ml_dtypes.bfloat16)


def _shard(x):
    # [2048, 1024] token-slice -> [128, 8, 2048] with [p, ic, t] = x[t, ic*128+p]
    return _bf16(x.T.reshape(8, 128, NS).transpose(1, 0, 2))


def _wlayout(w):
    # W [D, D] -> [128, 8, D] with [p, ic, d] = W[d, ic*128+p]
    return _bf16(w.T.reshape(8, 128, D).transpose(1, 0, 2))


def _run(nc, in_maps, trace=False, tmpdir=None):
    return run_bass_kernel_spmd(nc, in_maps, list(range(8)), trace=trace,
                                tmpdir=tmpdir)


def kernel(q, k, v, Wq, bq, Wk, bk, Wv, bv, Wo, bo, proj,
           _trace=False, _tmpdir=None):
    nc = _get_nc()

    q = np.asarray(q); k = np.asarray(k); v = np.asarray(v)
    Wq = np.asarray(Wq); Wk = np.asarray(Wk); Wv = np.asarray(Wv)
    Wo = np.asarray(Wo); proj = np.asarray(proj)
    bq = np.asarray(bq); bk = np.asarray(bk); bv = np.asarray(bv)
    bo = np.asarray(bo)

    pjt = _bf16(proj.T * SNORM)
    wqt = _wlayout(Wq)
    wkt = _wlayout(Wk)
    wvt = _wlayout(Wv)
    wot = _wlayout(Wo)
    idn = _bf16(np.eye(128, dtype=np.float32))
    mskc = np.zeros((128, 2), dtype=np.float32)
    mskc[0:64, 0] = CDIAG
    mskc[64:128, 1] = CDIAG
    mskc = _bf16(mskc)
    onea = _bf16(np.ones((128, 64), dtype=np.float32))
    oneb = _bf16(np.ones((128, 16, 16, 1), dtype=np.float32))
    bkcl = np.ascontiguousarray(bk.reshape(8, 128).T).astype(np.float32)
    # cq[m, h] = s * (bq_h . proj_m); exp(cq) scales kv_nat/ksum (exact)
    cq = (proj @ bq.reshape(H, HD).T) * SNORM          # [M, H]
    ecqt = np.ascontiguousarray(
        np.exp(cq).reshape(2, 128, H).transpose(1, 0, 2)).astype(np.float32)

    in_maps = []
    for c in range(8):
        b, s = divmod(c, 2)
        sl = slice(s * NS, (s + 1) * NS)
        in_maps.append({
            "qt": _shard(q[b, sl, :]),
            "kt": _shard(k[b, sl, :]),
            "vt": _shard(v[b, sl, :]),
            "wqt": wqt, "wkt": wkt, "wvt": wvt, "wot": wot,
            "pjt": pjt, "idn": idn, "mskc": mskc,
            "onea": onea, "oneb": oneb, "bkc": bkcl,
            "ecqt": ecqt,
        })

    res = _run(nc, in_maps, trace=_trace, tmpdir=_tmpdir)

    bo_eff = (bv @ Wo.T + bo).astype(np.float32)
    full = np.empty((B, N, D), dtype=np.float32)
    for c in range(8):
        b, s = divmod(c, 2)
        full[b, s * NS:(s + 1) * NS, :] = \
            res.results[c]["out"].astype(np.float32) + bo_eff

    if _trace:
        return full, res
    return full
